# revision 11
# baseline (speedup 1.0000x reference)
"""Trainium2 Bass kernel: image -> 2-photon Fock-state basis change.

The reference op is `out[fock_idx] = input_state` with `out` zeros elsewhere
(fock_idx injective), i.e. a pure row scatter [36864, 512] -> [73920, 512].

fock_idx has block structure: input rows [i*192, (i+1)*192) land on output
rows [start(i), start(i)+192) contiguously with start(i) quadratic in i, so
the scatter is 192 contiguous block copies plus zero fills — pure DMA work.

Sharding (fast path): split the *image rows* across the 8 cores — core k
copies blocks 24k..24k+23 with the full 512-wide batch, 384KB per flat
DRAM->DRAM dma_start, 24 instructions per core. The SPMD program stays
uniform by computing each core's output offsets from partition_id in
sequencer registers: local_row(j) = 192 + j*A - j(j-1)/2 with
A = 383 - 24*pid. Each core's output buffer is its slab of the Fock vector
(global rows [start(24k)-192, ...)); the host pastes slabs back together.

Zero rows are never written: the Bass runtime zero-initializes
ExternalOutput buffers (native path pre-zeros; the PJRT path feeds the NEFF
zero-filled buffers). kernel() validates this and repairs + warns if the
contract is ever violated.

A generic batch-sharded path (64 columns per core, one flat dma_start per
contiguous run, no partition_id math) handles any other injective fock_idx.
"""

import numpy as np

D1 = 192
D2 = 192
M = D1 + D2
IMG_DIM = D1 * D2            # 36864
FOCK_DIM = M * (M + 1) // 2  # 73920
BATCH = 512
N_CORES = 8
BS = BATCH // N_CORES        # batch-shard path: 64 columns per core

BPC = D1 // N_CORES          # row-shard path: 24 blocks per core
# uniform per-core output rows: 192 lead margin + largest slab
# (core 7: FOCK_DIM - start(168) = 23412 rows)
OUT_ROWS = 23604


def _fock_indices() -> np.ndarray:
    i = np.repeat(np.arange(D1), D2)
    j = np.tile(np.arange(D2), D1)
    q = D1 + j
    idx = i * M - i * (i - 1) // 2 + (q - i)
    return idx.astype(np.int32)


def _block_starts() -> np.ndarray:
    i = np.arange(D1, dtype=np.int64)
    return i * M - i * (i - 1) // 2 + (D1 - i)


# ---------------------------------------------------------------- planning


def _plan(fock_idx: np.ndarray):
    """Decompose the scatter into contiguous runs + zero intervals."""
    idx = np.asarray(fock_idx, dtype=np.int64).ravel()
    assert idx.shape[0] == IMG_DIM
    assert idx.min() >= 0 and idx.max() < FOCK_DIM
    assert np.unique(idx).size == IMG_DIM, "fock_idx must be injective"

    brk = np.nonzero(np.diff(idx) != 1)[0] + 1
    starts_in = np.concatenate([[0], brk])
    ends_in = np.concatenate([brk, [IMG_DIM]])
    runs = [(int(a), int(idx[a]), int(b - a)) for a, b in zip(starts_in, ends_in)]
    assert len(runs) <= 1024, f"scatter too fragmented: {len(runs)} runs"

    covered = np.zeros(FOCK_DIM, dtype=bool)
    covered[idx] = True
    d = np.diff(covered.astype(np.int8))
    zstarts = np.nonzero(d == -1)[0] + 1
    zends = np.nonzero(d == 1)[0] + 1
    if not covered[0]:
        zstarts = np.concatenate([[0], zstarts])
    if not covered[FOCK_DIM - 1]:
        zends = np.concatenate([zends, [FOCK_DIM]])
    zeros = [(int(a), int(b - a)) for a, b in zip(zstarts, zends)]
    assert sum(r[2] for r in runs) + sum(z[1] for z in zeros) == FOCK_DIM
    return runs, zeros


def _is_fock_pattern(runs) -> bool:
    if len(runs) != D1:
        return False
    starts = _block_starts()
    return all(
        a == i * D2 and ln == D2 and b == int(starts[i])
        for i, (a, b, ln) in enumerate(runs)
    )


# ---------------------------------------------------------------- programs


def _build_rowshard_program():
    """Raw bacc kernel (no Tile): 12 dynamic-offset DMAs per HWDGE engine,
    one semaphore wait per engine at the end. Avoids Tile's preamble/tail
    barriers and its 8-deep DMA in-flight cap — the HWDGE rings provide
    hardware backpressure."""
    import concourse.bacc as bacc
    import concourse.bass as bass
    from concourse import mybir

    nc = bacc.Bacc(
        "TRN2",
        debug=False,
        num_devices=N_CORES,
        enable_asserts=False,
        detect_race_conditions=False,
        monotonic_sem_count=0,
    )
    rows_in = BPC * D2  # 4608
    x = nc.dram_tensor(
        "x", [rows_in, BATCH], mybir.dt.float32, kind="ExternalInput"
    ).ap()
    y = nc.dram_tensor(
        "y", [OUT_ROWS, BATCH], mybir.dt.float32, kind="ExternalOutput"
    ).ap()

    with (
        nc.semaphore("dma_sp") as s_sp,
        nc.semaphore("dma_act") as s_act,
        nc.Block() as block,
    ):

        def body(eng, sem, jstart):
            n = 0
            if jstart == 0:
                # block 0 lands at local row 192 on every core — issue it
                # before the ~1.5us partition_id load
                eng.dma_start(out=y[D2 : 2 * D2, :], in_=x[0:D2, :]).then_inc(
                    sem, 16
                )
                n += 1
                jstart = 2
            pid = eng.partition_id()
            A = eng.snap(383 - pid * BPC)
            for j in range(jstart, BPC, 2):
                tj = j * (j - 1) // 2
                off_rows = A * j + (D2 - tj)
                eng.dma_start(
                    out=y[bass.ds(off_rows, D2), :],
                    in_=x[j * D2 : (j + 1) * D2, :],
                ).then_inc(sem, 16)
                n += 1
            eng.wait_ge(sem, 16 * n)

        @block.sync
        def _(sync):
            body(sync, s_sp, 0)

        @block.scalar
        def _(scalar):
            body(scalar, s_act, 1)

    nc.compile()
    return nc


def _build_batchshard_program(runs):
    import concourse.bacc as bacc
    import concourse.tile as tile
    from concourse import mybir

    nc = bacc.Bacc("TRN2", debug=False, num_devices=N_CORES)
    x = nc.dram_tensor("x", [IMG_DIM, BS], mybir.dt.float32, kind="ExternalInput").ap()
    y = nc.dram_tensor(
        "y", [FOCK_DIM, BS], mybir.dt.float32, kind="ExternalOutput"
    ).ap()

    with tile.TileContext(nc) as tc:
        engines = [nc.sync, nc.scalar]
        for k, (a, b, ln) in enumerate(runs):
            engines[k % 2].dma_start(out=y[b : b + ln, :], in_=x[a : a + ln, :])
    nc.compile()
    return nc


_cache = {}


def _get_program(fock_idx: np.ndarray):
    key = hash(np.asarray(fock_idx, dtype=np.int64).tobytes())
    if key not in _cache:
        runs, zeros = _plan(fock_idx)
        if _is_fock_pattern(runs):
            _cache[key] = ("row", _build_rowshard_program(), zeros)
        else:
            _cache[key] = ("batch", _build_batchshard_program(runs), zeros)
    return _cache[key]


# ---------------------------------------------------------------- execution


def _run(nc, in_maps, trace=False, tmpdir=None):
    from concourse import bass_utils

    kw = {"trace": True, "tmpdir": tmpdir} if trace else {}
    return bass_utils.run_bass_kernel_spmd(nc, in_maps, list(range(N_CORES)), **kw)


def _execute(x_full: np.ndarray, fock_idx: np.ndarray, trace=False, tmpdir=None):
    mode, nc, zeros = _get_program(fock_idx)

    if mode == "row":
        rows_in = BPC * D2
        in_maps = [
            {"x": x_full[c * rows_in : (c + 1) * rows_in]} for c in range(N_CORES)
        ]
        res = _run(nc, in_maps, trace, tmpdir)
        starts = _block_starts()
        out = np.zeros((FOCK_DIM, BATCH), dtype=np.float32)
        for k in range(N_CORES):
            g0 = int(starts[BPC * k])
            g1 = int(starts[BPC * (k + 1)]) if k < N_CORES - 1 else FOCK_DIM
            out[g0:g1] = res.results[k]["y"][D2 : D2 + (g1 - g0)]
    else:
        in_maps = [
            {"x": np.ascontiguousarray(x_full[:, c * BS : (c + 1) * BS])}
            for c in range(N_CORES)
        ]
        res = _run(nc, in_maps, trace, tmpdir)
        out = np.concatenate([res.results[c]["y"] for c in range(N_CORES)], axis=1)

    # The runtime hands the NEFF zero-initialized output buffers, so
    # unwritten rows must be zero. Validate; repair on the host if the
    # contract is ever violated (should never happen).
    bad = 0
    for r0, length in zeros:
        seg = out[r0 : r0 + length]
        if seg.any():
            bad += int(np.count_nonzero(seg))
            seg[:] = 0
    if bad:
        import sys

        print(
            f"WARNING: output buffer was not zero-initialized "
            f"({bad} nonzero elems in zero rows); repaired on host",
            file=sys.stderr,
        )
    return out, res


def kernel(**inputs) -> np.ndarray:
    x_full = np.ascontiguousarray(np.asarray(inputs["input_state"], dtype=np.float32))
    assert x_full.shape == (IMG_DIM, BATCH)
    fock_idx = inputs.get("fock_idx")
    fock_idx = (
        _fock_indices() if fock_idx is None else np.asarray(fock_idx, dtype=np.int64)
    )
    out, _ = _execute(x_full, fock_idx)
    return out.astype(np.float32, copy=False)


# revision 12
# speedup vs baseline: 1.0251x; 1.0251x over previous
"""Trainium2 Bass kernel: image -> 2-photon Fock-state basis change.

The reference op is `out[fock_idx] = input_state` with `out` zeros elsewhere
(fock_idx injective), i.e. a pure row scatter [36864, 512] -> [73920, 512].

fock_idx has block structure: input rows [i*192, (i+1)*192) land on output
rows [start(i), start(i)+192) contiguously with start(i) quadratic in i, so
the scatter is 192 contiguous block copies plus zero fills — pure DMA work.

Sharding (fast path): split the *image rows* across the 8 cores — core k
copies blocks 24k..24k+23 with the full 512-wide batch, 384KB per flat
DRAM->DRAM dma_start, 24 instructions per core. The SPMD program stays
uniform by computing each core's output offsets from partition_id in
sequencer registers: local_row(j) = 192 + j*A - j(j-1)/2 with
A = 383 - 24*pid. Each core's output buffer is its slab of the Fock vector
(global rows [start(24k)-192, ...)); the host pastes slabs back together.

Zero rows are never written: the Bass runtime zero-initializes
ExternalOutput buffers (native path pre-zeros; the PJRT path feeds the NEFF
zero-filled buffers). kernel() validates this and repairs + warns if the
contract is ever violated.

A generic batch-sharded path (64 columns per core, one flat dma_start per
contiguous run, no partition_id math) handles any other injective fock_idx.
"""

import numpy as np

D1 = 192
D2 = 192
M = D1 + D2
IMG_DIM = D1 * D2            # 36864
FOCK_DIM = M * (M + 1) // 2  # 73920
BATCH = 512
N_CORES = 8
BS = BATCH // N_CORES        # batch-shard path: 64 columns per core

BPC = D1 // N_CORES          # row-shard path: 24 blocks per core
# uniform per-core output rows: 192 lead margin + largest slab
# (core 7: FOCK_DIM - start(168) = 23412 rows)
OUT_ROWS = 23604


def _fock_indices() -> np.ndarray:
    i = np.repeat(np.arange(D1), D2)
    j = np.tile(np.arange(D2), D1)
    q = D1 + j
    idx = i * M - i * (i - 1) // 2 + (q - i)
    return idx.astype(np.int32)


def _block_starts() -> np.ndarray:
    i = np.arange(D1, dtype=np.int64)
    return i * M - i * (i - 1) // 2 + (D1 - i)


# ---------------------------------------------------------------- planning


def _plan(fock_idx: np.ndarray):
    """Decompose the scatter into contiguous runs + zero intervals."""
    idx = np.asarray(fock_idx, dtype=np.int64).ravel()
    assert idx.shape[0] == IMG_DIM
    assert idx.min() >= 0 and idx.max() < FOCK_DIM
    assert np.unique(idx).size == IMG_DIM, "fock_idx must be injective"

    brk = np.nonzero(np.diff(idx) != 1)[0] + 1
    starts_in = np.concatenate([[0], brk])
    ends_in = np.concatenate([brk, [IMG_DIM]])
    runs = [(int(a), int(idx[a]), int(b - a)) for a, b in zip(starts_in, ends_in)]
    assert len(runs) <= 1024, f"scatter too fragmented: {len(runs)} runs"

    covered = np.zeros(FOCK_DIM, dtype=bool)
    covered[idx] = True
    d = np.diff(covered.astype(np.int8))
    zstarts = np.nonzero(d == -1)[0] + 1
    zends = np.nonzero(d == 1)[0] + 1
    if not covered[0]:
        zstarts = np.concatenate([[0], zstarts])
    if not covered[FOCK_DIM - 1]:
        zends = np.concatenate([zends, [FOCK_DIM]])
    zeros = [(int(a), int(b - a)) for a, b in zip(zstarts, zends)]
    assert sum(r[2] for r in runs) + sum(z[1] for z in zeros) == FOCK_DIM
    return runs, zeros


def _is_fock_pattern(runs) -> bool:
    if len(runs) != D1:
        return False
    starts = _block_starts()
    return all(
        a == i * D2 and ln == D2 and b == int(starts[i])
        for i, (a, b, ln) in enumerate(runs)
    )


# ---------------------------------------------------------------- programs


def _build_rowshard_program():
    """Raw bacc kernel (no Tile): 12 DMAs per HWDGE engine, one semaphore
    wait per engine at the end. Skipping Tile removes its 8-deep DMA
    in-flight cap (each Tile DMA waits on the completion of the DMA eight
    back on its sem lane); here all DMAs queue immediately and the HWDGE
    rings provide hardware backpressure."""
    import concourse.bacc as bacc
    import concourse.bass as bass
    from concourse import mybir

    nc = bacc.Bacc(
        "TRN2",
        debug=False,
        num_devices=N_CORES,
        enable_asserts=False,
        detect_race_conditions=False,
        monotonic_sem_count=0,
    )
    rows_in = BPC * D2  # 4608
    x = nc.dram_tensor(
        "x", [rows_in, BATCH], mybir.dt.float32, kind="ExternalInput"
    ).ap()
    y = nc.dram_tensor(
        "y", [OUT_ROWS, BATCH], mybir.dt.float32, kind="ExternalOutput"
    ).ap()

    with (
        nc.semaphore("dma_sp") as s_sp,
        nc.semaphore("dma_act") as s_act,
        nc.Block() as block,
    ):

        def body(eng, sem, jstart):
            n = 0
            if jstart == 0:
                # block 0 lands at local row 192 on every core — issue it
                # before the ~1.5us partition_id load
                eng.dma_start(out=y[D2 : 2 * D2, :], in_=x[0:D2, :]).then_inc(
                    sem, 16
                )
                n += 1
                jstart = 2
            pid = eng.partition_id()
            A = eng.snap(383 - pid * BPC)
            for j in range(jstart, BPC, 2):
                tj = j * (j - 1) // 2
                off_rows = A * j + (D2 - tj)
                eng.dma_start(
                    out=y[bass.ds(off_rows, D2), :],
                    in_=x[j * D2 : (j + 1) * D2, :],
                ).then_inc(sem, 16)
                n += 1
            eng.wait_ge(sem, 16 * n)

        @block.sync
        def _(sync):
            body(sync, s_sp, 0)

        @block.scalar
        def _(scalar):
            body(scalar, s_act, 1)

    nc.compile()
    return nc


def _build_batchshard_program(runs):
    import concourse.bacc as bacc
    import concourse.tile as tile
    from concourse import mybir

    nc = bacc.Bacc("TRN2", debug=False, num_devices=N_CORES)
    x = nc.dram_tensor("x", [IMG_DIM, BS], mybir.dt.float32, kind="ExternalInput").ap()
    y = nc.dram_tensor(
        "y", [FOCK_DIM, BS], mybir.dt.float32, kind="ExternalOutput"
    ).ap()

    with tile.TileContext(nc) as tc:
        engines = [nc.sync, nc.scalar]
        for k, (a, b, ln) in enumerate(runs):
            engines[k % 2].dma_start(out=y[b : b + ln, :], in_=x[a : a + ln, :])
    nc.compile()
    return nc


_cache = {}


def _get_program(fock_idx: np.ndarray):
    key = hash(np.asarray(fock_idx, dtype=np.int64).tobytes())
    if key not in _cache:
        runs, zeros = _plan(fock_idx)
        if _is_fock_pattern(runs):
            _cache[key] = ("row", _build_rowshard_program(), zeros)
        else:
            _cache[key] = ("batch", _build_batchshard_program(runs), zeros)
    return _cache[key]


# ---------------------------------------------------------------- execution


def _run(nc, in_maps, trace=False, tmpdir=None):
    from concourse import bass_utils

    kw = {"trace": True, "tmpdir": tmpdir} if trace else {}
    return bass_utils.run_bass_kernel_spmd(nc, in_maps, list(range(N_CORES)), **kw)


def _execute(x_full: np.ndarray, fock_idx: np.ndarray, trace=False, tmpdir=None):
    mode, nc, zeros = _get_program(fock_idx)

    if mode == "row":
        rows_in = BPC * D2
        in_maps = [
            {"x": x_full[c * rows_in : (c + 1) * rows_in]} for c in range(N_CORES)
        ]
        res = _run(nc, in_maps, trace, tmpdir)
        starts = _block_starts()
        out = np.zeros((FOCK_DIM, BATCH), dtype=np.float32)
        for k in range(N_CORES):
            g0 = int(starts[BPC * k])
            g1 = int(starts[BPC * (k + 1)]) if k < N_CORES - 1 else FOCK_DIM
            out[g0:g1] = res.results[k]["y"][D2 : D2 + (g1 - g0)]
    else:
        in_maps = [
            {"x": np.ascontiguousarray(x_full[:, c * BS : (c + 1) * BS])}
            for c in range(N_CORES)
        ]
        res = _run(nc, in_maps, trace, tmpdir)
        out = np.concatenate([res.results[c]["y"] for c in range(N_CORES)], axis=1)

    # The runtime hands the NEFF zero-initialized output buffers, so
    # unwritten rows must be zero. Validate; repair on the host if the
    # contract is ever violated (should never happen).
    bad = 0
    for r0, length in zeros:
        seg = out[r0 : r0 + length]
        if seg.any():
            bad += int(np.count_nonzero(seg))
            seg[:] = 0
    if bad:
        import sys

        print(
            f"WARNING: output buffer was not zero-initialized "
            f"({bad} nonzero elems in zero rows); repaired on host",
            file=sys.stderr,
        )
    return out, res


def kernel(**inputs) -> np.ndarray:
    x_full = np.ascontiguousarray(np.asarray(inputs["input_state"], dtype=np.float32))
    assert x_full.shape == (IMG_DIM, BATCH)
    fock_idx = inputs.get("fock_idx")
    fock_idx = (
        _fock_indices() if fock_idx is None else np.asarray(fock_idx, dtype=np.int64)
    )
    out, _ = _execute(x_full, fock_idx)
    return out.astype(np.float32, copy=False)
